# revision 7
# baseline (speedup 1.0000x reference)
"""Trainium2 Bass kernel for the packed-sequence CrossEntropy-style loss.

Problem (hardcoded shapes): scores [8, 1024, 32000] f32, target [8, 1024] int,
lengths [8] int (descending, lengths[0] = 1024).

reference math per batch row b:
    lp   = log_softmax(scores[b], axis=-1)                    # [T, V]
    lp_t = lp[t, target[t]]            (0 where t >= len)     # [T]
    p    = exp(lp_t)                   (1 where t >= len)
    props[0] = 0.5 ; props[t] = 0.3*props[t-1] + 0.7*p[t-1]
    soft = softmax(props over valid t) * len  (0 at invalid)
    partial_b = sum_t lp_t * soft
loss = -sum_b partial_b / sum_b len_b

v3 sharding: TOKEN-parallel. Scores at padded positions (t >= len_b) never
reach the loss: soft==0 there and p at a padded t only feeds props at later,
also-padded positions. So only the sum(lengths) = N valid (b, t) tokens need
their [32000] vocab rows streamed. The global b-major valid-token stream is
split into 8 equal chunks of K = ceil(N/8) tokens; core c streams chunk c
(plus an H-token halo, see below) -- max-core HBM traffic drops from
T*V*4 = 131 MB to ~(K+H)*V*4 = ~76 MB.

Per-core math (one launch, no cross-core communication):
  - stream [128, chunk] f32 tiles, fused ACT exp+accumulate -> se (sum-exp
    per token), indirect-DMA gather of s_tgt = scores[t, target[t]].
  - the leaky-integrator scan runs over the core's local token stream in
    [NBLK, 128] layout (partition j = local token block). Row resets
    (t == 0) are handled EXACTLY by a per-element scan multiplier a that is
    0 at row starts (props[t0] = 0*prev + 0.5, carried in the data1 input).
    Cross-partition carry uses the baseline's trick: C_j = scan[j-1, 127]
    exactly (the cumulative a-product over 128 elements underflows to 0),
    applied as props += cumprod(a) * C_j, where cumprod(a) is a second
    (multiplicative) scan. cumprod(a) is 0 past a row start, which also
    correctly kills the carry across row boundaries.
  - chunks start mid-row: an H = 8 token halo (re-streamed by this core,
    scan entered with carry 0) bounds the props error by 0.3^H ~ 6.6e-5,
    absolutely negligible at the 2e-2 gate. A row start inside the halo
    makes the chunk exact.
  - per-row outputs: Se[b] = sum exp(props), Sz[b] = sum lp*exp(props) over
    the core's OWNED tokens of row b (halo/pad killed by a host-provided
    0/1 mask). 16 floats per core.
Host combine (gather only): Se_tot[b] = sum_c Se[c,b], same for Sz;
loss = -sum_b len_b * Sz_tot[b] / Se_tot[b] / sum(len).

Numerics notes (inherited from the verified v2 baseline):
  - No max-subtraction in the big log-sum-exp: inputs are N(0,1) so exp() is
    in range and the fp32 sum of 32000 such terms is accurate.
  - u[t] = 0.7*p[t] is computed as 0.7*exp(s_tgt)*(1/sumexp) via the exp
    bias input (exp(x + ln 0.7)), avoiding ACT's Ln.
  - lse = ln(sumexp) by 3 Newton steps on the exp table (seeded from the
    exponent bits), keeping the kernel exp-only: no activation-table
    switches.
  - props lies in [0, 1]: the tiny ragged softmax needs no max-subtraction.
  - pad lanes (local token slots past the real stream) get se patched to
    1.0, masks 0; everything stays finite and is masked out of the sums.
"""

import math
import numpy as np
from contextlib import ExitStack

import concourse.bass as bass
import concourse.bacc as bacc
import concourse.tile as tile
from concourse import mybir
from concourse.bass_utils import run_bass_kernel_spmd
from concourse.masks import make_identity

B, T, V = 8, 1024, 32000
P = 128            # SBUF partitions
N_CORES = 8
HALO = 8           # scan halo tokens; props error <= 0.3^HALO ~ 6.6e-5

# 16000 f32 = 64000 B per-partition descriptor (< 2^16 B limit)
CHUNKS_MAIN = [16000, 16000]
CHUNKS_LAST = [16000, 8000, 4000, 2000, 1000, 1000]
assert sum(CHUNKS_MAIN) == V and sum(CHUNKS_LAST) == V
MAXCH = max(len(CHUNKS_MAIN), len(CHUNKS_LAST))
MAXW = max(max(CHUNKS_MAIN), max(CHUNKS_LAST))

F32 = mybir.dt.float32
I32 = mybir.dt.int32
Alu = mybir.AluOpType
Act = mybir.ActivationFunctionType

LN07 = float(np.log(0.7))

SMALL_LOADS_ON_SCALAR = True  # small input loads ride the ACT HWDGE queue
STREAM_TWO_QUEUES = True      # alternate stream chunks across sync+tensor HWDGE queues
FIRST_BLOCK_HIGH_PRIO = True  # first block's stream DMAs issue in the preamble


def _emit(ctx: ExitStack, tc: "tile.TileContext", M, scores, gidx, aT, rs05,
          m01, wm, a1h, out):
    """M local token slots; NBLK = ceil(M/128) blocks, last one partial."""
    nc = tc.nc
    NBLK = (M + P - 1) // P
    LROWS = M - P * (NBLK - 1)   # rows in the last block

    data = ctx.enter_context(tc.tile_pool(name="data", bufs=3))
    singles = ctx.enter_context(tc.tile_pool(name="singles", bufs=1))
    psum = ctx.enter_context(tc.tile_pool(name="psum", bufs=1, space="PSUM"))

    # flat [M*V, 1] view of the local score chunk for the elementwise gather
    scores_flat = bass.AP(tensor=scores.tensor, offset=0, ap=[[1, M * V], [1, 1]])

    sums_all = singles.tile([P, NBLK, MAXCH], F32)    # per-(block, chunk) sum-exp
    idx_tile = singles.tile([P, NBLK], I32)
    starget = singles.tile([P, NBLK], F32)            # scores[s, target[s]]
    aT_t = singles.tile([NBLK, P], F32)               # scan multiplier (0 / 0.3)
    rs05_t = singles.tile([NBLK, P], F32)             # 0.5 at row starts
    m01_t = singles.tile([NBLK, P], F32)              # 1 where u_prev feeds uu
    wm_t = singles.tile([NBLK, B, P], F32)            # ownership one-hot per row
    a1 = singles.tile([NBLK, NBLK], F32)              # superdiagonal shift matrix

    # --- small input loads on the Activation HWDGE queue (keeps the Sync
    # queue free for the stream) ---
    small_q = nc.scalar if SMALL_LOADS_ON_SCALAR else nc.sync
    with tc.high_priority():
        small_q.dma_start(out=idx_tile[:, :], in_=gidx)
        small_q.dma_start(out=aT_t[:, :], in_=aT)
        small_q.dma_start(out=rs05_t[:, :], in_=rs05)
        small_q.dma_start(out=m01_t[:, :], in_=m01)
        small_q.dma_start(out=wm_t[:, :, :], in_=wm)
        small_q.dma_start(out=a1[:, :], in_=a1h)
    for j in range(NBLK):
        nc.gpsimd.indirect_dma_start(
            out=starget[:, j : j + 1],
            out_offset=None,
            in_=scores_flat,
            in_offset=bass.IndirectOffsetOnAxis(ap=idx_tile[:, j : j + 1], axis=0),
        )

    # --- dependency-free prep (scheduled under the streaming pass) ---
    identity = singles.tile([P, P], F32)
    make_identity(nc, identity[:, :])
    ones_T = singles.tile([NBLK, P], F32)
    nc.vector.memset(ones_T[:, :], 1.0)
    onesb = singles.tile([NBLK, 1], F32)
    nc.vector.memset(onesb[:, :], 1.0)

    exp_st = singles.tile([P, NBLK], F32)             # 0.7 * exp(s_target)
    se = singles.tile([P, NBLK], F32)                 # per-token sum-exp
    if LROWS < P:
        # pad lanes of the last block stay finite (masked later): memset the
        # whole column first, the partial reduce then overwrites valid lanes
        # (compute engines cannot address a partition range starting at 80).
        nc.vector.memset(se[:, NBLK - 1 : NBLK], 1.0)

    # ---- main streaming pass: [rows, chunk] f32 tiles, exp+accumulate ----
    stream_qs = [nc.sync, nc.scalar] if STREAM_TWO_QUEUES else [nc.sync]
    qi = 0
    for j in range(NBLK):
        rows = LROWS if j == NBLK - 1 else P
        col = 0
        chunks = CHUNKS_LAST if j == NBLK - 1 else CHUNKS_MAIN
        for c, w in enumerate(chunks):
            tl = data.tile([P, MAXW], F32, tag="tl")
            q = stream_qs[qi % len(stream_qs)]
            qi += 1
            if j == 0 and FIRST_BLOCK_HIGH_PRIO:
                with tc.high_priority():
                    q.dma_start(
                        out=tl[0:rows, 0:w],
                        in_=scores[j * P : j * P + rows, col : col + w],
                    )
            else:
                q.dma_start(
                    out=tl[0:rows, 0:w],
                    in_=scores[j * P : j * P + rows, col : col + w],
                )
            nc.scalar.activation(
                out=tl[0:rows, 0:w],
                in_=tl[0:rows, 0:w],
                func=Act.Exp,
                accum_out=sums_all[0:rows, j, c : c + 1],
            )
            col += w
        # per-block sum-exp reduce, scheduled under the stream
        nc.vector.reduce_sum(
            out=se[0:rows, j : j + 1],
            in_=sums_all[0:rows, j, 0 : len(chunks)],
            axis=mybir.AxisListType.X,
        )
    # exp_st = exp(s_tgt + ln 0.7). The bias tile carries a real data dep on
    # a late block's sum so the scheduler cannot queue this on the in-order
    # ACT queue before the stream's big exps (the gathers feeding starget
    # land late; an early-queued exp_st would stall the whole stream).
    ln07b = singles.tile([P, 1], F32)
    nc.vector.tensor_scalar(
        out=ln07b[:, :], in0=se[:, NBLK - 2 : NBLK - 1], scalar1=0.0, scalar2=LN07,
        op0=Alu.mult, op1=Alu.add,
    )
    nc.scalar.activation(
        out=exp_st[:, :], in_=starget[:, :], func=Act.Exp, bias=ln07b[:, 0:1]
    )

    # ---- tail: u = 0.7*exp(s_tgt)/se, lse = ln(se), lp = s_tgt - lse ----
    rse = singles.tile([P, NBLK], F32)
    nc.vector.reciprocal(out=rse[:, :], in_=se[:, :])
    u = singles.tile([P, NBLK], F32)
    nc.vector.tensor_tensor(out=u[:, :], in0=exp_st[:, :], in1=rse[:, :], op=Alu.mult)

    # transpose u -> [NBLK, 128] and start the scan path immediately
    pt_u = psum.tile([NBLK, P], F32)
    nc.tensor.transpose(out=pt_u[:, :], in_=u[:, :], identity=identity[:, :])
    u_T = singles.tile([NBLK, P], F32)
    nc.vector.tensor_copy(u_T[:, :], pt_u[:, :])

    # Newton-ln seed (runs concurrently with the scan path)
    # y0 = float(bits(se))*ln2/2^23 - 87.986236 (|err| < 0.044)
    lse = singles.tile([P, NBLK], F32)
    fbits = singles.tile([P, NBLK], F32)
    nc.vector.tensor_copy(fbits[:, :], se[:, :].bitcast(I32))
    nc.vector.tensor_scalar_mul(out=lse[:, :], in0=fbits[:, :], scalar1=8.262958405176314e-08)
    nc.vector.tensor_scalar_add(out=lse[:, :], in0=lse[:, :], scalar1=-87.98623657)

    # uu[s] = rs05[s] + m01[s] * u[s-1]  (u shifted one local slot)
    cv_ps = psum.tile([NBLK, 1], F32)
    nc.tensor.matmul(cv_ps[:, :], a1[:, :], u_T[:, P - 1 : P])
    cv = singles.tile([NBLK, 1], F32)
    nc.vector.tensor_copy(cv[:, :], cv_ps[:, :])
    uu = singles.tile([NBLK, P], F32)
    nc.vector.tensor_tensor(
        out=uu[:, 1:P], in0=m01_t[:, 1:P], in1=u_T[:, 0 : P - 1], op=Alu.mult
    )
    nc.vector.tensor_scalar(
        out=uu[:, 0:1], in0=m01_t[:, 0:1], scalar1=cv[:, 0:1], scalar2=None,
        op0=Alu.mult,
    )
    nc.vector.tensor_tensor(out=uu[:, :], in0=uu[:, :], in1=rs05_t[:, :], op=Alu.add)

    # Newton iteration 1: y += se*exp(-y) - 1
    ex = singles.tile([P, NBLK], F32)
    corr = singles.tile([P, NBLK], F32)
    nc.scalar.activation(out=ex[:, :], in_=lse[:, :], func=Act.Exp, scale=-1.0)
    nc.vector.tensor_tensor(out=corr[:, :], in0=se[:, :], in1=ex[:, :], op=Alu.mult)
    nc.vector.tensor_tensor(out=lse[:, :], in0=lse[:, :], in1=corr[:, :], op=Alu.add)
    nc.vector.tensor_scalar_add(out=lse[:, :], in0=lse[:, :], scalar1=-1.0)

    # block-local scans: scan0[j, i] = a[j,i]*state + uu[j,i]
    scan0 = singles.tile([NBLK, P], F32)
    nc.vector.tensor_tensor_scan(
        out=scan0[:, :],
        data0=aT_t[:, :],
        data1=uu[:, :],
        initial=0.0,
        op0=Alu.mult,
        op1=Alu.add,
    )
    # cumA[j, i] = prod_{i'<=i} a[j, i']  (0 past any row start)
    cumA = singles.tile([NBLK, P], F32)
    nc.vector.tensor_tensor_scan(
        out=cumA[:, :],
        data0=aT_t[:, :],
        data1=ones_T[:, :],
        initial=1.0,
        op0=Alu.mult,
        op1=Alu.mult,
    )

    # Newton iteration 2
    nc.scalar.activation(out=ex[:, :], in_=lse[:, :], func=Act.Exp, scale=-1.0)
    nc.vector.tensor_tensor(out=corr[:, :], in0=se[:, :], in1=ex[:, :], op=Alu.mult)
    nc.vector.tensor_tensor(out=lse[:, :], in0=lse[:, :], in1=corr[:, :], op=Alu.add)
    nc.vector.tensor_scalar_add(out=lse[:, :], in0=lse[:, :], scalar1=-1.0)

    # cross-partition scan carry: C[j] = scan0[j-1, 127] (exact; cumA over a
    # full block underflows to 0), props = scan0 + cumA * C
    c_ps = psum.tile([NBLK, 1], F32)
    nc.tensor.matmul(c_ps[:, :], a1[:, :], scan0[:, P - 1 : P])
    c_sb = singles.tile([NBLK, 1], F32)
    nc.vector.tensor_copy(c_sb[:, :], c_ps[:, :])
    props = singles.tile([NBLK, P], F32)
    nc.vector.tensor_scalar_mul(out=props[:, :], in0=cumA[:, :], scalar1=c_sb[:, 0:1])
    nc.vector.tensor_tensor(out=props[:, :], in0=props[:, :], in1=scan0[:, :], op=Alu.add)

    # Newton iteration 3
    nc.scalar.activation(out=ex[:, :], in_=lse[:, :], func=Act.Exp, scale=-1.0)
    nc.vector.tensor_tensor(out=corr[:, :], in0=se[:, :], in1=ex[:, :], op=Alu.mult)
    nc.vector.tensor_tensor(out=lse[:, :], in0=lse[:, :], in1=corr[:, :], op=Alu.add)
    nc.vector.tensor_scalar_add(out=lse[:, :], in0=lse[:, :], scalar1=-1.0)

    # lp = s_tgt - lse, transposed to [NBLK, 128]
    lp = singles.tile([P, NBLK], F32)
    nc.vector.tensor_tensor(out=lp[:, :], in0=starget[:, :], in1=lse[:, :], op=Alu.subtract)
    pt_lp = psum.tile([NBLK, P], F32)
    nc.tensor.transpose(out=pt_lp[:, :], in_=lp[:, :], identity=identity[:, :])
    lp_T = singles.tile([NBLK, P], F32)
    nc.vector.tensor_copy(lp_T[:, :], pt_lp[:, :])

    # e = exp(props); per-row masked sums Se[b], Sz[b]
    e_T = singles.tile([NBLK, P], F32)
    nc.scalar.activation(out=e_T[:, :], in_=props[:, :], func=Act.Exp)
    ze_T = singles.tile([NBLK, P], F32)
    nc.vector.tensor_tensor(out=ze_T[:, :], in0=lp_T[:, :], in1=e_T[:, :], op=Alu.mult)

    sums = singles.tile([NBLK, 2 * B], F32)
    emb = singles.tile([NBLK, P], F32)
    for b in range(B):
        nc.vector.tensor_tensor(
            out=emb[:, :], in0=e_T[:, :], in1=wm_t[:, b, :], op=Alu.mult
        )
        nc.vector.reduce_sum(
            out=sums[:, b : b + 1], in_=emb[:, :], axis=mybir.AxisListType.X
        )
        nc.vector.tensor_tensor(
            out=emb[:, :], in0=ze_T[:, :], in1=wm_t[:, b, :], op=Alu.mult
        )
        nc.vector.reduce_sum(
            out=sums[:, B + b : B + b + 1], in_=emb[:, :], axis=mybir.AxisListType.X
        )

    # cross-partition totals via ones-matmul -> [1, 16]
    fin_ps = psum.tile([1, 2 * B], F32)
    nc.tensor.matmul(fin_ps[:, :], onesb[:, :], sums[:, :])
    fin = singles.tile([1, 2 * B], F32)
    nc.vector.tensor_copy(fin[:, :], fin_ps[:, :])
    nc.sync.dma_start(out=out, in_=fin[:, :])


_program_cache: dict[int, object] = {}


def build_program(M):
    if M in _program_cache:
        return _program_cache[M]
    NBLK = (M + P - 1) // P
    nc = bacc.Bacc(
        "TRN2", target_bir_lowering=False, debug=False, num_devices=N_CORES
    )
    scores = nc.dram_tensor("scores", [M, V], F32, kind="ExternalInput").ap()
    gidx = nc.dram_tensor("gidx", [P, NBLK], I32, kind="ExternalInput").ap()
    aT = nc.dram_tensor("aT", [NBLK, P], F32, kind="ExternalInput").ap()
    rs05 = nc.dram_tensor("rs05", [NBLK, P], F32, kind="ExternalInput").ap()
    m01 = nc.dram_tensor("m01", [NBLK, P], F32, kind="ExternalInput").ap()
    wm = nc.dram_tensor("wm", [NBLK, B, P], F32, kind="ExternalInput").ap()
    a1h = nc.dram_tensor("a1h", [NBLK, NBLK], F32, kind="ExternalInput").ap()
    out = nc.dram_tensor("out", [1, 2 * B], F32, kind="ExternalOutput").ap()

    with tile.TileContext(nc) as tc, ExitStack() as ctx:
        _emit(ctx, tc, M, scores, gidx, aT, rs05, m01, wm, a1h, out)
    nc.compile()
    _program_cache[M] = nc
    return nc


def make_in_maps(scores, target, lengths, M, K, N):
    scores = np.asarray(scores, dtype=np.float32)
    target = np.asarray(target).astype(np.int64)
    lengths = np.asarray(lengths).astype(np.int64)
    NBLK = (M + P - 1) // P
    MP = NBLK * P

    # global b-major valid-token stream
    b_of = np.repeat(np.arange(B, dtype=np.int64), lengths)       # [N]
    t_of = np.concatenate([np.arange(l, dtype=np.int64) for l in lengths])

    # superdiagonal shift matrix (shared across cores)
    a1 = np.zeros((NBLK, NBLK), dtype=np.float32)
    for j in range(1, NBLK):
        a1[j - 1, j] = 1.0

    in_maps = []
    for c in range(N_CORES):
        start = c * K - HALO
        g = start + np.arange(MP, dtype=np.int64)                 # global slot ids
        valid = (g >= 0) & (g < N) & (np.arange(MP) < M)
        gc = np.clip(g, 0, N - 1)
        bb = np.where(valid, b_of[gc], 0)
        tt = np.where(valid, t_of[gc], 0)

        owned = valid & (g >= c * K) & (g < min((c + 1) * K, N))
        row_start = valid & (tt == 0)
        slot0 = np.zeros(MP, dtype=bool)
        slot0[0] = True
        # scan multiplier: 0 at row starts, slot 0, and invalid slots
        a_vec = np.where(valid & ~row_start & ~slot0, 0.3, 0.0).astype(np.float32)
        rs_vec = np.where(row_start, 0.5, 0.0).astype(np.float32)
        m_vec = np.where(valid & ~row_start & ~slot0, 1.0, 0.0).astype(np.float32)

        # ownership one-hot [slot, b]
        w = np.zeros((MP, B), dtype=np.float32)
        w[np.arange(MP)[owned], bb[owned]] = 1.0

        # gather index: slot*V + target[b, t] (0 for invalid slots)
        gi = np.where(
            valid, np.arange(MP, dtype=np.int64) * V + target[bb, tt], 0
        ).astype(np.int32)

        in_maps.append(
            {
                "scores": np.ascontiguousarray(scores[bb[:M], tt[:M]]),
                "gidx": np.ascontiguousarray(
                    gi.reshape(NBLK, P).T.astype(np.int32)
                ),
                "aT": a_vec.reshape(NBLK, P),
                "rs05": rs_vec.reshape(NBLK, P),
                "m01": m_vec.reshape(NBLK, P),
                "wm": np.ascontiguousarray(
                    w.reshape(NBLK, P, B).transpose(0, 2, 1)
                ),
                "a1h": a1,
            }
        )
    return in_maps


def finish(outs, lengths):
    lengths = np.asarray(lengths).astype(np.int64)
    se_tot = np.zeros(B, dtype=np.float64)
    sz_tot = np.zeros(B, dtype=np.float64)
    for o in outs:
        se_tot += o[0, 0:B].astype(np.float64)
        sz_tot += o[0, B : 2 * B].astype(np.float64)
    total = float(lengths.sum())
    partials = lengths.astype(np.float64) * sz_tot / se_tot
    return np.float32(-float(partials.sum()) / total)


def kernel(scores, target, lengths, _trace: bool = False):
    lengths_np = np.asarray(lengths).astype(np.int64)
    N = int(lengths_np.sum())
    K = math.ceil(N / N_CORES)
    M = K + HALO
    nc = build_program(M)
    in_maps = make_in_maps(scores, target, lengths_np, M, K, N)
    res = run_bass_kernel_spmd(nc, in_maps, core_ids=list(range(N_CORES)), trace=_trace)
    outs = [np.asarray(res.results[i]["out"]) for i in range(N_CORES)]
    loss = finish(outs, lengths_np)
    if _trace:
        kernel.last_results = res
    return loss


# revision 11
# speedup vs baseline: 1.1207x; 1.1207x over previous
"""Trainium2 Bass kernel for the packed-sequence CrossEntropy-style loss.

Problem (hardcoded shapes): scores [8, 1024, 32000] f32, target [8, 1024] int,
lengths [8] int (descending, lengths[0] = 1024).

reference math per batch row b:
    lp   = log_softmax(scores[b], axis=-1)                    # [T, V]
    lp_t = lp[t, target[t]]            (0 where t >= len)     # [T]
    p    = exp(lp_t)                   (1 where t >= len)
    props[0] = 0.5 ; props[t] = 0.3*props[t-1] + 0.7*p[t-1]
    soft = softmax(props over valid t) * len  (0 at invalid)
    partial_b = sum_t lp_t * soft
loss = -sum_b partial_b / sum_b len_b

v3 sharding: TOKEN-parallel. Scores at padded positions (t >= len_b) never
reach the loss: soft==0 there and p at a padded t only feeds props at later,
also-padded positions. So only the sum(lengths) = N valid (b, t) tokens need
their [32000] vocab rows streamed. The global b-major valid-token stream is
split into 8 equal chunks of K = ceil(N/8) tokens; core c streams chunk c
(plus an H-token halo, see below) -- max-core HBM traffic drops from
T*V*4 = 131 MB to ~(K+H)*V*4 = ~76 MB.

Per-core math (one launch, no cross-core communication):
  - stream [128, chunk] f32 tiles, fused ACT exp+accumulate -> se (sum-exp
    per token), indirect-DMA gather of s_tgt = scores[t, target[t]].
  - the leaky-integrator scan runs over the core's local token stream in
    [NBLK, 128] layout (partition j = local token block). Row resets
    (t == 0) are handled EXACTLY by a per-element scan multiplier a that is
    0 at row starts (props[t0] = 0*prev + 0.5, carried in the data1 input).
    Cross-partition carry uses the baseline's trick: C_j = scan[j-1, 127]
    exactly (the cumulative a-product over 128 elements underflows to 0),
    applied as props += cumprod(a) * C_j, where cumprod(a) is a second
    (multiplicative) scan. cumprod(a) is 0 past a row start, which also
    correctly kills the carry across row boundaries.
  - chunks start mid-row: an H = 8 token halo (re-streamed by this core,
    scan entered with carry 0) bounds the props error by 0.3^H ~ 6.6e-5,
    absolutely negligible at the 2e-2 gate. A row start inside the halo
    makes the chunk exact.
  - per-row outputs: Se[b] = sum exp(props), Sz[b] = sum lp*exp(props) over
    the core's OWNED tokens of row b (halo/pad killed by a host-provided
    0/1 mask). 16 floats per core.
Host combine (gather only): Se_tot[b] = sum_c Se[c,b], same for Sz;
loss = -sum_b len_b * Sz_tot[b] / Se_tot[b] / sum(len).

Numerics notes (inherited from the verified v2 baseline):
  - No max-subtraction in the big log-sum-exp: inputs are N(0,1) so exp() is
    in range and the fp32 sum of 32000 such terms is accurate.
  - u[t] = 0.7*p[t] is computed as 0.7*exp(s_tgt)*(1/sumexp) via the exp
    bias input (exp(x + ln 0.7)), avoiding ACT's Ln.
  - lse = ln(sumexp) by 3 Newton steps on the exp table (seeded from the
    exponent bits), keeping the kernel exp-only: no activation-table
    switches.
  - props lies in [0, 1]: the tiny ragged softmax needs no max-subtraction.
  - pad lanes (local token slots past the real stream) get se patched to
    1.0, masks 0; everything stays finite and is masked out of the sums.
"""

import math
import numpy as np
from contextlib import ExitStack

import concourse.bass as bass
import concourse.bacc as bacc
import concourse.tile as tile
from concourse import mybir
from concourse.bass_utils import run_bass_kernel_spmd
from concourse.masks import make_identity

B, T, V = 8, 1024, 32000
P = 128            # SBUF partitions
N_CORES = 8
HALO = 8           # scan halo tokens; props error <= 0.3^HALO ~ 6.6e-5

# 16000 f32 = 64000 B per-partition descriptor (< 2^16 B limit).
# The last block tapers so ScalarE drains ~0.7 us behind the final DMA; only
# 3 chunks (the taper region is descriptor-rate-limited, so fewer, larger
# descriptors beat the old 6-step taper).
CHUNKS_MAIN = [16000, 16000]
CHUNKS_LAST = [16368, 14608, 1024]
assert sum(CHUNKS_MAIN) == V and sum(CHUNKS_LAST) == V
MAXCH = max(len(CHUNKS_MAIN), len(CHUNKS_LAST))
MAXW = max(max(CHUNKS_MAIN), max(CHUNKS_LAST))

F32 = mybir.dt.float32
I32 = mybir.dt.int32
Alu = mybir.AluOpType
Act = mybir.ActivationFunctionType

LN07 = float(np.log(0.7))

SMALL_LOADS_ON_SCALAR = True  # small input loads ride the ACT HWDGE queue
STREAM_TWO_QUEUES = False     # measured regression: ACT sequencer issues its
                              # queue's dma_starts between 10us EXPs -> bursty
FIRST_BLOCK_HIGH_PRIO = False # measured: does not move the stream start


def _emit(ctx: ExitStack, tc: "tile.TileContext", M, scores, gidx, aT, rs05,
          m01, wm, a1h, out):
    """M local token slots; NBLK = ceil(M/128) blocks, last one partial."""
    nc = tc.nc
    NBLK = (M + P - 1) // P
    LROWS = M - P * (NBLK - 1)   # rows in the last block

    data = ctx.enter_context(tc.tile_pool(name="data", bufs=3))
    singles = ctx.enter_context(tc.tile_pool(name="singles", bufs=1))
    psum = ctx.enter_context(tc.tile_pool(name="psum", bufs=1, space="PSUM"))

    # flat [M*V, 1] view of the local score chunk for the elementwise gather
    scores_flat = bass.AP(tensor=scores.tensor, offset=0, ap=[[1, M * V], [1, 1]])

    sums_all = singles.tile([P, NBLK, MAXCH], F32)    # per-(block, chunk) sum-exp
    idx_tile = singles.tile([P, NBLK], I32)
    starget = singles.tile([P, NBLK], F32)            # scores[s, target[s]]
    aT_t = singles.tile([NBLK, P], F32)               # scan multiplier (0 / 0.3)
    rs05_t = singles.tile([NBLK, P], F32)             # 0.5 at row starts
    m01_t = singles.tile([NBLK, P], F32)              # 1 where u_prev feeds uu
    wm_t = singles.tile([NBLK, B, P], F32)            # ownership one-hot per row
    a1 = singles.tile([NBLK, NBLK], F32)              # superdiagonal shift matrix

    # --- small input loads on the Activation HWDGE queue (keeps the Sync
    # queue free for the stream) ---
    small_q = nc.scalar if SMALL_LOADS_ON_SCALAR else nc.sync
    with tc.high_priority():
        small_q.dma_start(out=idx_tile[:, :], in_=gidx)
        small_q.dma_start(out=aT_t[:, :], in_=aT)
        small_q.dma_start(out=rs05_t[:, :], in_=rs05)
        small_q.dma_start(out=m01_t[:, :], in_=m01)
        small_q.dma_start(out=wm_t[:, :, :], in_=wm)
        small_q.dma_start(out=a1[:, :], in_=a1h)
    for j in range(NBLK):
        nc.gpsimd.indirect_dma_start(
            out=starget[:, j : j + 1],
            out_offset=None,
            in_=scores_flat,
            in_offset=bass.IndirectOffsetOnAxis(ap=idx_tile[:, j : j + 1], axis=0),
        )

    # --- dependency-free prep (scheduled under the streaming pass) ---
    identity = singles.tile([P, P], F32)
    make_identity(nc, identity[:, :])
    ones_T = singles.tile([NBLK, P], F32)
    nc.vector.memset(ones_T[:, :], 1.0)
    onesb = singles.tile([NBLK, 1], F32)
    nc.vector.memset(onesb[:, :], 1.0)

    exp_st = singles.tile([P, NBLK], F32)             # 0.7 * exp(s_target)
    se = singles.tile([P, NBLK], F32)                 # per-token sum-exp
    if LROWS < P:
        # pad lanes of the last block stay finite (masked later): memset the
        # whole column first, the partial reduce then overwrites valid lanes
        # (compute engines cannot address a partition range starting at 80).
        nc.vector.memset(se[:, NBLK - 1 : NBLK], 1.0)

    # ---- main streaming pass: [rows, chunk] f32 tiles, exp+accumulate ----
    stream_qs = [nc.sync, nc.scalar] if STREAM_TWO_QUEUES else [nc.sync]
    qi = 0
    for j in range(NBLK):
        rows = LROWS if j == NBLK - 1 else P
        col = 0
        chunks = CHUNKS_LAST if j == NBLK - 1 else CHUNKS_MAIN
        for c, w in enumerate(chunks):
            tl = data.tile([P, MAXW], F32, tag="tl")
            q = stream_qs[qi % len(stream_qs)]
            qi += 1
            if j == 0 and FIRST_BLOCK_HIGH_PRIO:
                with tc.high_priority():
                    q.dma_start(
                        out=tl[0:rows, 0:w],
                        in_=scores[j * P : j * P + rows, col : col + w],
                    )
            else:
                q.dma_start(
                    out=tl[0:rows, 0:w],
                    in_=scores[j * P : j * P + rows, col : col + w],
                )
            nc.scalar.activation(
                out=tl[0:rows, 0:w],
                in_=tl[0:rows, 0:w],
                func=Act.Exp,
                accum_out=sums_all[0:rows, j, c : c + 1],
            )
            col += w
        # per-block sum-exp reduce, scheduled under the stream
        nc.vector.reduce_sum(
            out=se[0:rows, j : j + 1],
            in_=sums_all[0:rows, j, 0 : len(chunks)],
            axis=mybir.AxisListType.X,
        )
    # exp_st = exp(s_tgt + ln 0.7). The bias tile carries a real data dep on
    # a late block's sum so the scheduler cannot queue this on the in-order
    # ACT queue before the stream's big exps (the gathers feeding starget
    # land late; an early-queued exp_st would stall the whole stream).
    ln07b = singles.tile([P, 1], F32)
    nc.vector.tensor_scalar(
        out=ln07b[:, :], in0=se[:, NBLK - 2 : NBLK - 1], scalar1=0.0, scalar2=LN07,
        op0=Alu.mult, op1=Alu.add,
    )
    nc.scalar.activation(
        out=exp_st[:, :], in_=starget[:, :], func=Act.Exp, bias=ln07b[:, 0:1]
    )

    # ---- tail: u = 0.7*exp(s_tgt)/se, lse = ln(se), lp = s_tgt - lse ----
    rse = singles.tile([P, NBLK], F32)
    nc.vector.reciprocal(out=rse[:, :], in_=se[:, :])
    u = singles.tile([P, NBLK], F32)
    nc.vector.tensor_tensor(out=u[:, :], in0=exp_st[:, :], in1=rse[:, :], op=Alu.mult)

    # transpose u -> [NBLK, 128] and start the scan path immediately
    pt_u = psum.tile([NBLK, P], F32)
    nc.tensor.transpose(out=pt_u[:, :], in_=u[:, :], identity=identity[:, :])
    u_T = singles.tile([NBLK, P], F32)
    nc.vector.tensor_copy(u_T[:, :], pt_u[:, :])

    # Newton-ln seed (runs concurrently with the scan path)
    # y0 = float(bits(se))*ln2/2^23 - 87.986236 (|err| < 0.044)
    lse = singles.tile([P, NBLK], F32)
    fbits = singles.tile([P, NBLK], F32)
    nc.vector.tensor_copy(fbits[:, :], se[:, :].bitcast(I32))
    nc.vector.tensor_scalar_mul(out=lse[:, :], in0=fbits[:, :], scalar1=8.262958405176314e-08)
    nc.vector.tensor_scalar_add(out=lse[:, :], in0=lse[:, :], scalar1=-87.98623657)

    # uu[s] = rs05[s] + m01[s] * u[s-1]  (u shifted one local slot)
    cv_ps = psum.tile([NBLK, 1], F32)
    nc.tensor.matmul(cv_ps[:, :], a1[:, :], u_T[:, P - 1 : P])
    cv = singles.tile([NBLK, 1], F32)
    nc.vector.tensor_copy(cv[:, :], cv_ps[:, :])
    uu = singles.tile([NBLK, P], F32)
    nc.vector.tensor_tensor(
        out=uu[:, 1:P], in0=m01_t[:, 1:P], in1=u_T[:, 0 : P - 1], op=Alu.mult
    )
    nc.vector.tensor_scalar(
        out=uu[:, 0:1], in0=m01_t[:, 0:1], scalar1=cv[:, 0:1], scalar2=None,
        op0=Alu.mult,
    )
    nc.vector.tensor_tensor(out=uu[:, :], in0=uu[:, :], in1=rs05_t[:, :], op=Alu.add)

    # Newton iteration 1: y += se*exp(-y) - 1
    ex = singles.tile([P, NBLK], F32)
    corr = singles.tile([P, NBLK], F32)
    nc.scalar.activation(out=ex[:, :], in_=lse[:, :], func=Act.Exp, scale=-1.0)
    nc.vector.tensor_tensor(out=corr[:, :], in0=se[:, :], in1=ex[:, :], op=Alu.mult)
    nc.vector.tensor_tensor(out=lse[:, :], in0=lse[:, :], in1=corr[:, :], op=Alu.add)
    nc.vector.tensor_scalar_add(out=lse[:, :], in0=lse[:, :], scalar1=-1.0)

    # block-local scans: scan0[j, i] = a[j,i]*state + uu[j,i]
    scan0 = singles.tile([NBLK, P], F32)
    nc.vector.tensor_tensor_scan(
        out=scan0[:, :],
        data0=aT_t[:, :],
        data1=uu[:, :],
        initial=0.0,
        op0=Alu.mult,
        op1=Alu.add,
    )
    # cumA[j, i] = prod_{i'<=i} a[j, i']  (0 past any row start)
    cumA = singles.tile([NBLK, P], F32)
    nc.vector.tensor_tensor_scan(
        out=cumA[:, :],
        data0=aT_t[:, :],
        data1=ones_T[:, :],
        initial=1.0,
        op0=Alu.mult,
        op1=Alu.mult,
    )

    # Newton iteration 2
    nc.scalar.activation(out=ex[:, :], in_=lse[:, :], func=Act.Exp, scale=-1.0)
    nc.vector.tensor_tensor(out=corr[:, :], in0=se[:, :], in1=ex[:, :], op=Alu.mult)
    nc.vector.tensor_tensor(out=lse[:, :], in0=lse[:, :], in1=corr[:, :], op=Alu.add)
    nc.vector.tensor_scalar_add(out=lse[:, :], in0=lse[:, :], scalar1=-1.0)

    # cross-partition scan carry: C[j] = scan0[j-1, 127] (exact; cumA over a
    # full block underflows to 0), props = scan0 + cumA * C
    c_ps = psum.tile([NBLK, 1], F32)
    nc.tensor.matmul(c_ps[:, :], a1[:, :], scan0[:, P - 1 : P])
    c_sb = singles.tile([NBLK, 1], F32)
    nc.vector.tensor_copy(c_sb[:, :], c_ps[:, :])
    props = singles.tile([NBLK, P], F32)
    nc.vector.tensor_scalar_mul(out=props[:, :], in0=cumA[:, :], scalar1=c_sb[:, 0:1])
    nc.vector.tensor_tensor(out=props[:, :], in0=props[:, :], in1=scan0[:, :], op=Alu.add)

    # Newton iteration 3
    nc.scalar.activation(out=ex[:, :], in_=lse[:, :], func=Act.Exp, scale=-1.0)
    nc.vector.tensor_tensor(out=corr[:, :], in0=se[:, :], in1=ex[:, :], op=Alu.mult)
    nc.vector.tensor_tensor(out=lse[:, :], in0=lse[:, :], in1=corr[:, :], op=Alu.add)
    nc.vector.tensor_scalar_add(out=lse[:, :], in0=lse[:, :], scalar1=-1.0)

    # lp = s_tgt - lse, transposed to [NBLK, 128]
    lp = singles.tile([P, NBLK], F32)
    nc.vector.tensor_tensor(out=lp[:, :], in0=starget[:, :], in1=lse[:, :], op=Alu.subtract)
    pt_lp = psum.tile([NBLK, P], F32)
    nc.tensor.transpose(out=pt_lp[:, :], in_=lp[:, :], identity=identity[:, :])
    lp_T = singles.tile([NBLK, P], F32)
    nc.vector.tensor_copy(lp_T[:, :], pt_lp[:, :])

    # e = exp(props); per-row masked sums Se[b], Sz[b] via broadcast-AP
    # tensor_tensor over [NBLK, B, P] + one X-axis reduce each
    e_T = singles.tile([NBLK, P], F32)
    nc.scalar.activation(out=e_T[:, :], in_=props[:, :], func=Act.Exp)

    def _bcast(t):
        ap = t[:, :]
        return bass.AP(tensor=ap.tensor, offset=ap.offset,
                       ap=[ap.ap[0], [0, B], ap.ap[1]])

    sums = singles.tile([NBLK, 2 * B], F32)
    em_big = singles.tile([NBLK, B, P], F32)
    nc.vector.tensor_tensor(
        out=em_big[:, :, :], in0=_bcast(e_T), in1=wm_t[:, :, :], op=Alu.mult
    )
    nc.vector.reduce_sum(
        out=sums[:, 0:B], in_=em_big[:, :, :], axis=mybir.AxisListType.X
    )
    # in-place: em_big *= lp (broadcast); ordered after the first reduce by
    # the tile framework's RAW/WAR tracking (SBUF budget is tight)
    nc.vector.tensor_tensor(
        out=em_big[:, :, :], in0=_bcast(lp_T), in1=em_big[:, :, :], op=Alu.mult
    )
    nc.vector.reduce_sum(
        out=sums[:, B : 2 * B], in_=em_big[:, :, :], axis=mybir.AxisListType.X
    )

    # cross-partition totals via ones-matmul -> [1, 16]
    fin_ps = psum.tile([1, 2 * B], F32)
    nc.tensor.matmul(fin_ps[:, :], onesb[:, :], sums[:, :])
    fin = singles.tile([1, 2 * B], F32)
    nc.vector.tensor_copy(fin[:, :], fin_ps[:, :])
    nc.sync.dma_start(out=out, in_=fin[:, :])


_program_cache: dict[int, object] = {}


def build_program(M):
    if M in _program_cache:
        return _program_cache[M]
    NBLK = (M + P - 1) // P
    nc = bacc.Bacc(
        "TRN2", target_bir_lowering=False, debug=False, num_devices=N_CORES
    )
    scores = nc.dram_tensor("scores", [M, V], F32, kind="ExternalInput").ap()
    gidx = nc.dram_tensor("gidx", [P, NBLK], I32, kind="ExternalInput").ap()
    aT = nc.dram_tensor("aT", [NBLK, P], F32, kind="ExternalInput").ap()
    rs05 = nc.dram_tensor("rs05", [NBLK, P], F32, kind="ExternalInput").ap()
    m01 = nc.dram_tensor("m01", [NBLK, P], F32, kind="ExternalInput").ap()
    wm = nc.dram_tensor("wm", [NBLK, B, P], F32, kind="ExternalInput").ap()
    a1h = nc.dram_tensor("a1h", [NBLK, NBLK], F32, kind="ExternalInput").ap()
    out = nc.dram_tensor("out", [1, 2 * B], F32, kind="ExternalOutput").ap()

    with tile.TileContext(nc) as tc, ExitStack() as ctx:
        _emit(ctx, tc, M, scores, gidx, aT, rs05, m01, wm, a1h, out)
    nc.compile()
    _program_cache[M] = nc
    return nc


def make_in_maps(scores, target, lengths, M, K, N):
    scores = np.asarray(scores, dtype=np.float32)
    target = np.asarray(target).astype(np.int64)
    lengths = np.asarray(lengths).astype(np.int64)
    NBLK = (M + P - 1) // P
    MP = NBLK * P

    # global b-major valid-token stream
    b_of = np.repeat(np.arange(B, dtype=np.int64), lengths)       # [N]
    t_of = np.concatenate([np.arange(l, dtype=np.int64) for l in lengths])

    # superdiagonal shift matrix (shared across cores)
    a1 = np.zeros((NBLK, NBLK), dtype=np.float32)
    for j in range(1, NBLK):
        a1[j - 1, j] = 1.0

    in_maps = []
    for c in range(N_CORES):
        start = c * K - HALO
        g = start + np.arange(MP, dtype=np.int64)                 # global slot ids
        valid = (g >= 0) & (g < N) & (np.arange(MP) < M)
        gc = np.clip(g, 0, N - 1)
        bb = np.where(valid, b_of[gc], 0)
        tt = np.where(valid, t_of[gc], 0)

        owned = valid & (g >= c * K) & (g < min((c + 1) * K, N))
        row_start = valid & (tt == 0)
        slot0 = np.zeros(MP, dtype=bool)
        slot0[0] = True
        # scan multiplier: 0 at row starts, slot 0, and invalid slots
        a_vec = np.where(valid & ~row_start & ~slot0, 0.3, 0.0).astype(np.float32)
        rs_vec = np.where(row_start, 0.5, 0.0).astype(np.float32)
        m_vec = np.where(valid & ~row_start & ~slot0, 1.0, 0.0).astype(np.float32)

        # ownership one-hot [slot, b]
        w = np.zeros((MP, B), dtype=np.float32)
        w[np.arange(MP)[owned], bb[owned]] = 1.0

        # gather index: slot*V + target[b, t] (0 for invalid slots)
        gi = np.where(
            valid, np.arange(MP, dtype=np.int64) * V + target[bb, tt], 0
        ).astype(np.int32)

        in_maps.append(
            {
                "scores": np.ascontiguousarray(scores[bb[:M], tt[:M]]),
                "gidx": np.ascontiguousarray(
                    gi.reshape(NBLK, P).T.astype(np.int32)
                ),
                "aT": a_vec.reshape(NBLK, P),
                "rs05": rs_vec.reshape(NBLK, P),
                "m01": m_vec.reshape(NBLK, P),
                "wm": np.ascontiguousarray(
                    w.reshape(NBLK, P, B).transpose(0, 2, 1)
                ),
                "a1h": a1,
            }
        )
    return in_maps


def finish(outs, lengths):
    lengths = np.asarray(lengths).astype(np.int64)
    se_tot = np.zeros(B, dtype=np.float64)
    sz_tot = np.zeros(B, dtype=np.float64)
    for o in outs:
        se_tot += o[0, 0:B].astype(np.float64)
        sz_tot += o[0, B : 2 * B].astype(np.float64)
    total = float(lengths.sum())
    partials = lengths.astype(np.float64) * sz_tot / se_tot
    return np.float32(-float(partials.sum()) / total)


def kernel(scores, target, lengths, _trace: bool = False):
    lengths_np = np.asarray(lengths).astype(np.int64)
    N = int(lengths_np.sum())
    K = math.ceil(N / N_CORES)
    M = K + HALO
    nc = build_program(M)
    in_maps = make_in_maps(scores, target, lengths_np, M, K, N)
    res = run_bass_kernel_spmd(nc, in_maps, core_ids=list(range(N_CORES)), trace=_trace)
    outs = [np.asarray(res.results[i]["out"]) for i in range(N_CORES)]
    loss = finish(outs, lengths_np)
    if _trace:
        kernel.last_results = res
    return loss
